# revision 28
# baseline (speedup 1.0000x reference)
"""HGNN message passing (gather + segment_sum + residual) on 8 trn2 cores.

out = x + segment_sum(x[src_idx], dst_idx, num_segments=N)

Strategy (node-sharded accumulation, no collectives):
  - dst nodes sharded across 8 cores (12500 nodes each); each core owns the
    edges targeting its node range and produces its [12500, 128] output slice.
  - Nodes are processed in GROUPS of 5 blocks of 125. Edges of a group are
    bucketed by src//25000 (4 buckets, int16 gather-offset reach) and packed
    tightly, block-major, with a -1 tail the Q7 gather kernel never touches
    (the count register carries the exact edge count); the 4 buckets' gathers
    run CONCURRENTLY on the 4 Q7 core-pairs (one SWDGE queue each), and the
    per-edge Q7 descriptor-generation rate (~8ns/edge/pair) is the kernel's
    hard bottleneck -- bigger groups amortize the per-instruction overhead
    (GROUP=5 beat 4; 10 loses more to SBUF-forced shallow buffering than it
    gains).
  - bf16 x rows (256B) are fetched with gpsimd dma_gather; 12 stage buffers
    (3 groups) plus 3 rotating count-register sets keep the in-order GpSimd
    engine fed (a shared register set would add a write-after-read edge
    from each group's reg_loads to the previous group's gathers, locking
    the pairs into group lockstep).
  - Per (group, bucket), ONE fused DVE is_equal builds the one-hot matrices
    from a host-prebuilt dstv image whose values are duplicated pairwise
    (innermost stride-1 pairs on both operands unlock the DVE 2x_1p perf
    mode -- twice the one-hot build rate of a stride-0 broadcast compare,
    and with no on-device expansion step the compare depends on nothing but
    its tile pool). The segment-sum is a sum of bf16 one-hot matmuls
    accumulated in PSUM, one PSUM tile per block. A chunk whose slot range
    can straddle a block boundary gets one matmul per candidate block (the
    host writes -5 into the other blocks' dstv so the one-hot is zero
    there). The residual enters the same PSUM accumulation as an
    identity-matrix matmul against the bf16 x row block.
  - The idx image streams through a small pool one (group, bucket) tile at
    a time instead of one huge upfront load (group 0's tiles load before
    the large dstv const so the first gathers start early); stage-tile
    zeroing runs on the otherwise-idle ACT engine at startup; residual
    loads land once per GROUP (one strided DMA for 5 blocks) on the sync
    engine's queue; output blocks are staged through SBUF by the Scalar
    (ACT) engine and written once per GROUP from ACT's HWDGE queue.

All cores run one SPMD program; per-core data differences live entirely in
the input tensors. The matmul template (chunk count, per-block chunk spans)
is computed from the actual edge data at build time, uniform across cores.
"""
import os

import numpy as np

N_NODES = 100000
D = 128
N_CORES = 8
NODES_PER_CORE = N_NODES // N_CORES  # 12500
BLOCK = 125
NBLOCKS = NODES_PER_CORE // BLOCK  # 100
if os.environ.get("KERNEL_NBLOCKS"):  # debug-only scale-down (multiple of GROUP)
    NBLOCKS = int(os.environ["KERNEL_NBLOCKS"])
GROUP = 5
NGROUPS = NBLOCKS // GROUP
NBKT = 4
SRC_CHUNK = N_NODES // NBKT  # 25000
NGATH = NGROUPS * NBKT  # gathers per core
STAGE_BUFS = 12

_cached = {}


def _build_program(chunks_p, los, his):
    """chunks_p: slots per (group,bucket) gather / 128; block h of a group
    only ever has edges in chunks [los[h], his[h]] (host-verified)."""
    from concourse import bacc, mybir, library_config
    import concourse.tile as tile

    capp = chunks_p * 128
    spans = [his[h] - los[h] + 1 for h in range(GROUP)]
    col_base = np.concatenate([[0], np.cumsum(spans)]).astype(int)
    nc_pb = int(col_base[-1])  # dstv/onehot columns per (group, bucket)
    idx_cols = NGATH * (capp // 16)
    gidx_cols = NBKT * (capp // 16)  # idx columns per group

    nc = bacc.Bacc("TRN2", debug=False, num_swdge_queues=4)
    f32 = mybir.dt.float32
    bf16 = mybir.dt.bfloat16
    x_t = nc.dram_tensor("x", [N_NODES, D], bf16, kind="ExternalInput")
    xresb_t = nc.dram_tensor("xresb", [NBLOCKS * BLOCK, D], bf16, kind="ExternalInput")
    idx_t = nc.dram_tensor("idx", [128, idx_cols], mybir.dt.int16, kind="ExternalInput")
    cnt_t = nc.dram_tensor("cnt", [1, NGATH], mybir.dt.int32, kind="ExternalInput")
    # dstv with every column value duplicated pairwise (see module docstring)
    dstv_t = nc.dram_tensor(
        "dstv", [128, NGATH * nc_pb * 2], bf16, kind="ExternalInput"
    )
    iota_t = nc.dram_tensor("iota", [128, 128], bf16, kind="ExternalInput")
    ident_t = nc.dram_tensor("ident", [128, BLOCK], bf16, kind="ExternalInput")
    out_t = nc.dram_tensor("out", [NBLOCKS * BLOCK, D], f32, kind="ExternalOutput")

    with tile.TileContext(nc) as tc:
        with (
            tc.tile_pool(name="consts", bufs=1) as constp,
            tc.tile_pool(name="idxp", bufs=16) as idxp,
            tc.tile_pool(name="stage", bufs=STAGE_BUFS) as stagep,
            tc.tile_pool(name="oh", bufs=8) as ohp,
            tc.tile_pool(name="psum", bufs=8, space="PSUM") as psump,
            tc.tile_pool(name="resid", bufs=3) as residp,
            tc.tile_pool(name="osb", bufs=3) as osbp,
        ):
            nc.gpsimd.load_library(library_config.mlp)
            # cnt + first idx group load first: the first gathers depend on
            # them, everything else can trail
            cnt_sb = constp.tile([1, NGATH], mybir.dt.int32)
            nc.sync.dma_start(cnt_sb[:], cnt_t[:])
            # 3 rotating register sets: reg_load(g+1,k) would otherwise have
            # a register WAR on gather(g,k), serializing the Q7 pair
            # pipeline into lockstep groups
            NREGSET = 3
            cnt_regs = [
                nc.gpsimd.alloc_register(f"cnt{r}_{k}")
                for r in range(NREGSET)
                for k in range(NBKT)
            ]
            # group 0's idx image loads before the big dstv const so the
            # first gathers are not stuck behind it on the sync queue
            idx_g0 = []
            for k in range(NBKT):
                t = idxp.tile([128, capp // 16], mybir.dt.int16)
                nc.sync.dma_start(t[:], idx_t[:, k * (capp // 16) : (k + 1) * (capp // 16)])
                idx_g0.append(t)
            dstv_sb = constp.tile([128, NGATH * nc_pb * 2], bf16)
            nc.sync.dma_start(dstv_sb[:], dstv_t[:])
            iota_sb = constp.tile([128, 128], bf16)
            nc.sync.dma_start(iota_sb[:], iota_t[:])
            ident_sb = constp.tile([128, BLOCK], bf16)
            nc.sync.dma_start(ident_sb[:], ident_t[:])

            # zero staging once: stale SBUF may hold NaN bit patterns, and
            # NaN * 0 would poison the PSUM accumulation. Run on ACT, which
            # is idle at startup (DVE must not delay the first IS_EQs, and
            # gpsimd must not delay the first gathers).
            for _ in range(STAGE_BUFS):
                stage = stagep.tile([128, chunks_p, D], bf16)
                nc.scalar.memzero(stage[:])

            iota_b = (
                iota_sb[:]
                .rearrange("p (a b) -> p a b", b=2)
                .unsqueeze(1)
                .broadcast_to([128, nc_pb, 64, 2])
            )

            for grp in range(NGROUPS):
                # per-bucket idx tiles: each gather depends only on its own
                # quarter of the group's idx image, so the first gathers can
                # start as soon as their slice lands
                if grp == 0:
                    idx_gk = idx_g0
                else:
                    idx_gk = []
                    for k in range(NBKT):
                        t = idxp.tile([128, capp // 16], mybir.dt.int16)
                        base = grp * gidx_cols + k * (capp // 16)
                        nc.sync.dma_start(t[:], idx_t[:, base : base + capp // 16])
                        idx_gk.append(t)
                stages_g = []
                ohbs_g = []
                # one batched register load for the group's 4 counts, then
                # the 4 gathers dispatch back-to-back on the engine (each
                # runs on its own Q7 pair; interleaved reg_loads would add
                # dispatch serialization between the pairs' starts)
                gregs = [cnt_regs[(grp % NREGSET) * NBKT + k] for k in range(NBKT)]
                nc.gpsimd.reg_load(gregs, cnt_sb[:1, grp * NBKT : (grp + 1) * NBKT])
                for k in range(NBKT):
                    stage = stagep.tile([128, chunks_p, D], bf16)
                    nc.gpsimd.dma_gather(
                        stage[:],
                        x_t[k * SRC_CHUNK : (k + 1) * SRC_CHUNK, :],
                        idx_gk[k][:],
                        capp,
                        gregs[k],
                        D,
                        single_packet=False,
                        queue_num=k,
                    )
                    stages_g.append(stage)
                    g = grp * NBKT + k
                    ohb = ohp.tile([128, nc_pb, 128], bf16)
                    nc.vector.tensor_tensor(
                        ohb[:].rearrange("p c (a b) -> p c a b", b=2),
                        dstv_sb[:, g * nc_pb * 2 : (g + 1) * nc_pb * 2]
                        .rearrange("p (c b) -> p c b", b=2)
                        .unsqueeze(2)
                        .broadcast_to([128, nc_pb, 64, 2]),
                        iota_b,
                        mybir.AluOpType.is_equal,
                    )
                    ohbs_g.append(ohb)

                # one strided DMA loads the whole group's residual rows
                resid = residp.tile([BLOCK, GROUP, D], bf16)
                nc.sync.dma_start(
                    resid[:],
                    xresb_t[grp * GROUP * BLOCK : (grp + 1) * GROUP * BLOCK].rearrange(
                        "(h p) d -> p h d", h=GROUP
                    ),
                )
                osb = osbp.tile([BLOCK, GROUP, D], f32)
                for h in range(GROUP):
                    psum = psump.tile([BLOCK, D], f32, space="PSUM")
                    nc.tensor.matmul(
                        out=psum[:],
                        lhsT=ident_sb[:BLOCK, :],
                        rhs=resid[:, h, :],
                        start=True,
                        stop=False,
                    )
                    ncols_h = his[h] - los[h] + 1
                    for k in range(NBKT):
                        for i in range(ncols_h):
                            nc.tensor.matmul(
                                out=psum[:],
                                lhsT=ohbs_g[k][:, int(col_base[h]) + i, :BLOCK],
                                rhs=stages_g[k][:, los[h] + i, :],
                                start=False,
                                stop=(k == NBKT - 1 and i == ncols_h - 1),
                            )
                    nc.scalar.copy(osb[:, h, :], psum[:])
                # one strided DMA writes the whole group's output, issued
                # from the ACT engine's HWDGE queue
                nc.scalar.dma_start(
                    out_t[grp * GROUP * BLOCK : (grp + 1) * GROUP * BLOCK].rearrange(
                        "(h p) d -> p h d", h=GROUP
                    ),
                    osb[:],
                )

    nc.compile()
    return nc


def _preprocess(src, dst):
    """Pack edges into tight per-(group,bucket) gather regions; build the idx
    image, exact counts, and the dstv one-hot source columns."""
    src = src.astype(np.int64)
    dst = dst.astype(np.int64)
    if NBLOCKS < NODES_PER_CORE // BLOCK:  # debug: drop edges past the cut
        keep = (dst % NODES_PER_CORE) // BLOCK < NBLOCKS
        src, dst = src[keep], dst[keep]
    E = src.shape[0]
    core = dst // NODES_PER_CORE
    blk = (dst % NODES_PER_CORE) // BLOCK
    half = blk % GROUP
    grp = blk // GROUP
    dloc = (dst % NODES_PER_CORE) % BLOCK
    bkt = src // SRC_CHUNK
    sloc = src % SRC_CHUNK
    region = (core * NGROUPS + grp) * NBKT + bkt  # gather region id
    tot_reg = N_CORES * NGATH

    key = region * GROUP + half
    # sort each (region, block) segment by source row: the gather then walks
    # HBM in increasing address order, improving DRAM row locality
    order = np.lexsort((sloc, key))
    ks = key[order]
    counts_h = np.bincount(key, minlength=tot_reg * GROUP)
    starts_h = np.zeros(tot_reg * GROUP + 1, np.int64)
    np.cumsum(counts_h, out=starts_h[1:])
    within = np.arange(E, dtype=np.int64) - starts_h[ks]

    ch = counts_h.reshape(tot_reg, GROUP)
    pref = np.zeros((tot_reg, GROUP + 1), np.int64)
    np.cumsum(ch, axis=1, out=pref[:, 1:])
    cnt_tot = pref[:, GROUP]
    # slot within region: block-major packing
    slot = np.empty(E, np.int64)
    slot[order] = within + pref[ks // GROUP, ks % GROUP]

    # build-time template parameters (uniform across cores by construction)
    chunks_p = int(np.ceil(cnt_tot.max() / 128))
    los = tuple(int(v) for v in (pref[:, :GROUP] // 128).min(axis=0))
    his = tuple(
        int(v) for v in (np.maximum(pref[:, 1:] - 1, pref[:, :GROUP]) // 128).max(axis=0)
    )
    capp = chunks_p * 128

    idx_arr = np.full(tot_reg * capp, -1, np.int16)
    idx_arr[region * capp + slot] = sloc.astype(np.int16)
    cnt_arr = np.ascontiguousarray(
        cnt_tot.reshape(N_CORES, 1, NGATH).astype(np.int32)
    )

    # dstv columns: per region, block h covers chunks [los[h], his[h]];
    # -5 where the slot isn't the column's block
    spans = [his[h] - los[h] + 1 for h in range(GROUP)]
    col_base = np.concatenate([[0], np.cumsum(spans)]).astype(np.int64)
    nc_pb = int(col_base[-1])
    chunk = slot // 128
    pos = slot % 128
    colidx = col_base[half] + (chunk - np.asarray(los)[half])
    dcol = region * nc_pb + colidx
    dst_arr = np.full((tot_reg * nc_pb, 128), -5.0, np.float32)
    dst_arr[dcol, pos] = dloc.astype(np.float32)

    # idx: logical slot i of a gather -> partition i%16, col i//16; tile 16->128
    idx_sb = (
        idx_arr.reshape(N_CORES, NGATH, capp // 16, 16)
        .transpose(0, 3, 1, 2)
        .reshape(N_CORES, 16, NGATH * (capp // 16))
    )
    idx_sb = np.ascontiguousarray(np.tile(idx_sb, (1, 8, 1)))
    # dstv: [core, 128 partitions, cols], then duplicate each column value
    # pairwise along the innermost axis for the DVE 2x_1p compare
    dst_sb = dst_arr.reshape(N_CORES, NGATH * nc_pb, 128).transpose(0, 2, 1)
    dst_sb = np.ascontiguousarray(np.repeat(dst_sb, 2, axis=2))
    return idx_sb, dst_sb, cnt_arr, chunks_p, los, his


def _run(x, src_idx, dst_idx, trace=False, trace_kwargs=None):
    import ml_dtypes
    from concourse import bass_utils

    bf16 = ml_dtypes.bfloat16
    x = np.ascontiguousarray(np.asarray(x, dtype=np.float32))
    idx_sb, dst_sb, cnt_arr, chunks_p, los, his = _preprocess(
        np.asarray(src_idx), np.asarray(dst_idx)
    )
    dst_sb = dst_sb.astype(bf16)

    tkey = (chunks_p, los, his)
    if _cached.get("key") != tkey:
        _cached["nc"] = _build_program(chunks_p, los, his)
        _cached["key"] = tkey
    nc = _cached["nc"]

    x_bf = x.astype(bf16)
    iota = np.tile(np.arange(128, dtype=np.float32), (128, 1)).astype(bf16)
    ident = np.zeros((128, BLOCK), dtype=np.float32)
    ident[np.arange(BLOCK), np.arange(BLOCK)] = 1.0
    ident = ident.astype(bf16)
    in_maps = []
    for c in range(N_CORES):
        in_maps.append(
            {
                "x": x_bf,
                "xresb": x_bf[c * NODES_PER_CORE : c * NODES_PER_CORE + NBLOCKS * BLOCK],
                "idx": idx_sb[c],
                "cnt": cnt_arr[c],
                "dstv": dst_sb[c],
                "iota": iota,
                "ident": ident,
            }
        )
    kw = dict(trace_kwargs or {})
    res = bass_utils.run_bass_kernel_spmd(
        nc, in_maps, core_ids=list(range(N_CORES)), trace=trace, **kw
    )
    out = np.concatenate([r["out"] for r in res.results], axis=0)
    return out, res


def kernel(x, src_idx, dst_idx):
    out, _ = _run(x, src_idx, dst_idx)
    return out


# revision 30
# speedup vs baseline: 1.1280x; 1.1280x over previous
"""HGNN message passing (gather + segment_sum + residual) on 8 trn2 cores.

out = x + segment_sum(x[src_idx], dst_idx, num_segments=N)

Strategy (node-sharded accumulation, no collectives):
  - dst nodes sharded across 8 cores (12500 nodes each); each core owns the
    edges targeting its node range and produces its [12500, 128] output slice.
  - Nodes are processed in GROUPS of 5 blocks of 125. Edges of a group are
    bucketed by src//25000 (4 buckets, int16 gather-offset reach) and packed
    tightly, block-major, with a -1 tail the Q7 gather kernel never touches
    (the count register carries the exact edge count); the 4 buckets' gathers
    run CONCURRENTLY on the 4 Q7 core-pairs (one SWDGE queue each), and the
    per-edge Q7 descriptor-generation rate (~8ns/edge/pair) is the kernel's
    hard bottleneck -- bigger groups amortize the per-instruction overhead
    (GROUP=5 beat 4; 10 loses more to SBUF-forced shallow buffering than it
    gains).
  - bf16 x rows (256B) are fetched with gpsimd dma_gather; 12 stage buffers
    (3 groups) plus 3 rotating count-register sets keep the in-order GpSimd
    engine fed (a shared register set would add a write-after-read edge
    from each group's reg_loads to the previous group's gathers, locking
    the pairs into group lockstep).
  - Per (group, bucket), ONE fused DVE is_equal builds the one-hot matrices
    from a host-prebuilt dstv image whose values are duplicated pairwise
    (innermost stride-1 pairs on both operands unlock the DVE 2x_1p perf
    mode -- twice the one-hot build rate of a stride-0 broadcast compare,
    and with no on-device expansion step the compare depends on nothing but
    its tile pool). The segment-sum is a sum of bf16 one-hot matmuls
    accumulated in PSUM, one PSUM tile per block. A chunk whose slot range
    can straddle a block boundary gets one matmul per candidate block (the
    host writes -5 into the other blocks' dstv so the one-hot is zero
    there). The residual enters the same PSUM accumulation as an
    identity-matrix matmul against the bf16 x row block.
  - The idx image streams through a small pool one (group, bucket) tile at
    a time instead of one huge upfront load (group 0's tiles load before
    the large dstv const so the first gathers start early); stage-tile
    zeroing runs on the otherwise-idle ACT engine at startup; residual
    loads land once per GROUP (one strided DMA for 5 blocks) on the sync
    engine's queue; output blocks are staged through SBUF by the Scalar
    (ACT) engine and written once per GROUP from ACT's HWDGE queue.

All cores run one SPMD program; per-core data differences live entirely in
the input tensors. The matmul template (chunk count, per-block chunk spans)
is computed from the actual edge data at build time, uniform across cores.
"""
import os

import numpy as np

N_NODES = 100000
D = 128
N_CORES = 8
NODES_PER_CORE = N_NODES // N_CORES  # 12500
BLOCK = 125
NBLOCKS = NODES_PER_CORE // BLOCK  # 100
if os.environ.get("KERNEL_NBLOCKS"):  # debug-only scale-down (multiple of GROUP)
    NBLOCKS = int(os.environ["KERNEL_NBLOCKS"])
GROUP = 5
NGROUPS = NBLOCKS // GROUP
NBKT = 4
# Overlapping 32768-row gather windows (int16 offset reach). ~31% of srcs
# fall in an overlap and can be assigned to either adjacent queue; the host
# uses them to equalize the 4 Q7 pairs' per-(core,group) edge counts, since
# the group makespan is the max of the 4 concurrent descriptor-gen times.
WBASE = (0, 22411, 44822, 67232)
WSIZE = 32768
NGATH = NGROUPS * NBKT  # gathers per core
STAGE_BUFS = 12

_cached = {}


def _build_program(chunks_p, pad_lo, los, his):
    """chunks_p: slots per (group,bucket) gather / 128; block h of a group
    only ever has edges in chunks [los[h], his[h]] (host-verified)."""
    from concourse import bacc, mybir, library_config
    import concourse.tile as tile

    capp = chunks_p * 128
    spans = [his[h] - los[h] + 1 for h in range(GROUP)]
    col_base = np.concatenate([[0], np.cumsum(spans)]).astype(int)
    nc_pb = int(col_base[-1])  # dstv/onehot columns per (group, bucket)
    idx_cols = NGATH * (capp // 16)
    gidx_cols = NBKT * (capp // 16)  # idx columns per group

    nc = bacc.Bacc("TRN2", debug=False, num_swdge_queues=4)
    f32 = mybir.dt.float32
    bf16 = mybir.dt.bfloat16
    x_t = nc.dram_tensor("x", [N_NODES, D], bf16, kind="ExternalInput")
    xresb_t = nc.dram_tensor("xresb", [NBLOCKS * BLOCK, D], bf16, kind="ExternalInput")
    idx_t = nc.dram_tensor("idx", [128, idx_cols], mybir.dt.int16, kind="ExternalInput")
    cnt_t = nc.dram_tensor("cnt", [1, NGATH], mybir.dt.int32, kind="ExternalInput")
    # dstv with every column value duplicated pairwise (see module docstring)
    dstv_t = nc.dram_tensor(
        "dstv", [128, NGATH * nc_pb * 2], bf16, kind="ExternalInput"
    )
    iota_t = nc.dram_tensor("iota", [128, 128], bf16, kind="ExternalInput")
    ident_t = nc.dram_tensor("ident", [128, BLOCK], bf16, kind="ExternalInput")
    out_t = nc.dram_tensor("out", [NBLOCKS * BLOCK, D], f32, kind="ExternalOutput")

    with tile.TileContext(nc) as tc:
        with (
            tc.tile_pool(name="consts", bufs=1) as constp,
            tc.tile_pool(name="idxp", bufs=16) as idxp,
            tc.tile_pool(name="stage", bufs=STAGE_BUFS) as stagep,
            tc.tile_pool(name="oh", bufs=8) as ohp,
            tc.tile_pool(name="psum", bufs=8, space="PSUM") as psump,
            tc.tile_pool(name="resid", bufs=3) as residp,
            tc.tile_pool(name="osb", bufs=3) as osbp,
        ):
            nc.gpsimd.load_library(library_config.mlp)
            # cnt + first idx group load first: the first gathers depend on
            # them, everything else can trail
            cnt_sb = constp.tile([1, NGATH], mybir.dt.int32)
            nc.sync.dma_start(cnt_sb[:], cnt_t[:])
            # 3 rotating register sets: reg_load(g+1,k) would otherwise have
            # a register WAR on gather(g,k), serializing the Q7 pair
            # pipeline into lockstep groups
            NREGSET = 3
            cnt_regs = [
                nc.gpsimd.alloc_register(f"cnt{r}_{k}")
                for r in range(NREGSET)
                for k in range(NBKT)
            ]
            # group 0's idx image loads before the big dstv const so the
            # first gathers are not stuck behind it on the sync queue
            idx_g0 = []
            for k in range(NBKT):
                t = idxp.tile([128, capp // 16], mybir.dt.int16)
                nc.sync.dma_start(t[:], idx_t[:, k * (capp // 16) : (k + 1) * (capp // 16)])
                idx_g0.append(t)
            dstv_sb = constp.tile([128, NGATH * nc_pb * 2], bf16)
            nc.sync.dma_start(dstv_sb[:], dstv_t[:])
            iota_sb = constp.tile([128, 128], bf16)
            nc.sync.dma_start(iota_sb[:], iota_t[:])
            ident_sb = constp.tile([128, BLOCK], bf16)
            nc.sync.dma_start(ident_sb[:], ident_t[:])

            # zero staging once: stale SBUF may hold NaN bit patterns, and
            # NaN * 0 would poison the PSUM accumulation. Run on ACT, which
            # is idle at startup (DVE must not delay the first IS_EQs, and
            # gpsimd must not delay the first gathers).
            # only chunks >= pad_lo can ever hold boot-stale (possibly
            # NaN-patterned) slots that a matmul later reads through a zero
            # one-hot; chunks below are overwritten by every gather
            for _ in range(STAGE_BUFS):
                stage = stagep.tile([128, chunks_p, D], bf16)
                nc.scalar.memzero(stage[:, pad_lo:, :])

            iota_b = (
                iota_sb[:]
                .rearrange("p (a b) -> p a b", b=2)
                .unsqueeze(1)
                .broadcast_to([128, nc_pb, 64, 2])
            )

            for grp in range(NGROUPS):
                # per-bucket idx tiles: each gather depends only on its own
                # quarter of the group's idx image, so the first gathers can
                # start as soon as their slice lands
                if grp == 0:
                    idx_gk = idx_g0
                else:
                    idx_gk = []
                    for k in range(NBKT):
                        t = idxp.tile([128, capp // 16], mybir.dt.int16)
                        base = grp * gidx_cols + k * (capp // 16)
                        nc.sync.dma_start(t[:], idx_t[:, base : base + capp // 16])
                        idx_gk.append(t)
                stages_g = []
                ohbs_g = []
                # one batched register load for the group's 4 counts, then
                # the 4 gathers dispatch back-to-back on the engine (each
                # runs on its own Q7 pair; interleaved reg_loads would add
                # dispatch serialization between the pairs' starts)
                gregs = [cnt_regs[(grp % NREGSET) * NBKT + k] for k in range(NBKT)]
                nc.gpsimd.reg_load(gregs, cnt_sb[:1, grp * NBKT : (grp + 1) * NBKT])
                for k in range(NBKT):
                    stage = stagep.tile([128, chunks_p, D], bf16)
                    nc.gpsimd.dma_gather(
                        stage[:],
                        x_t[WBASE[k] : WBASE[k] + WSIZE, :],
                        idx_gk[k][:],
                        capp,
                        gregs[k],
                        D,
                        single_packet=False,
                        queue_num=k,
                    )
                    stages_g.append(stage)
                    g = grp * NBKT + k
                    ohb = ohp.tile([128, nc_pb, 128], bf16)
                    nc.vector.tensor_tensor(
                        ohb[:].rearrange("p c (a b) -> p c a b", b=2),
                        dstv_sb[:, g * nc_pb * 2 : (g + 1) * nc_pb * 2]
                        .rearrange("p (c b) -> p c b", b=2)
                        .unsqueeze(2)
                        .broadcast_to([128, nc_pb, 64, 2]),
                        iota_b,
                        mybir.AluOpType.is_equal,
                    )
                    ohbs_g.append(ohb)

                # one strided DMA loads the whole group's residual rows
                resid = residp.tile([BLOCK, GROUP, D], bf16)
                nc.sync.dma_start(
                    resid[:],
                    xresb_t[grp * GROUP * BLOCK : (grp + 1) * GROUP * BLOCK].rearrange(
                        "(h p) d -> p h d", h=GROUP
                    ),
                )
                osb = osbp.tile([BLOCK, GROUP, D], f32)
                for h in range(GROUP):
                    psum = psump.tile([BLOCK, D], f32, space="PSUM")
                    nc.tensor.matmul(
                        out=psum[:],
                        lhsT=ident_sb[:BLOCK, :],
                        rhs=resid[:, h, :],
                        start=True,
                        stop=False,
                    )
                    ncols_h = his[h] - los[h] + 1
                    for k in range(NBKT):
                        for i in range(ncols_h):
                            nc.tensor.matmul(
                                out=psum[:],
                                lhsT=ohbs_g[k][:, int(col_base[h]) + i, :BLOCK],
                                rhs=stages_g[k][:, los[h] + i, :],
                                start=False,
                                stop=(k == NBKT - 1 and i == ncols_h - 1),
                            )
                    nc.scalar.copy(osb[:, h, :], psum[:])
                # one strided DMA writes the whole group's output, issued
                # from the ACT engine's HWDGE queue
                nc.scalar.dma_start(
                    out_t[grp * GROUP * BLOCK : (grp + 1) * GROUP * BLOCK].rearrange(
                        "(h p) d -> p h d", h=GROUP
                    ),
                    osb[:],
                )

    nc.compile()
    return nc


def _preprocess(src, dst):
    """Pack edges into tight per-(group,bucket) gather regions; build the idx
    image, exact counts, and the dstv one-hot source columns."""
    src = src.astype(np.int64)
    dst = dst.astype(np.int64)
    if NBLOCKS < NODES_PER_CORE // BLOCK:  # debug: drop edges past the cut
        keep = (dst % NODES_PER_CORE) // BLOCK < NBLOCKS
        src, dst = src[keep], dst[keep]
    E = src.shape[0]
    core = dst // NODES_PER_CORE
    blk = (dst % NODES_PER_CORE) // BLOCK
    half = blk % GROUP
    grp = blk // GROUP
    dloc = (dst % NODES_PER_CORE) % BLOCK
    # balanced bucket assignment over the overlapping windows: srcs covered
    # by two windows are waterfilled so each (core, group) cell's 4 bucket
    # counts are as equal as possible (the Q7 pairs run in lockstep per
    # group, so the makespan is the max of the 4 counts)
    W = np.asarray(WBASE, np.int64)
    lo_excl = np.array([0, WBASE[0] + WSIZE, WBASE[1] + WSIZE, WBASE[2] + WSIZE])
    bkt = np.searchsorted(W[1:], src, side="right")  # lowest covering window
    oclass = np.where(
        (src >= W[1]) & (src < lo_excl[1]), 0,
        np.where((src >= W[2]) & (src < lo_excl[2]), 1,
                 np.where((src >= W[3]) & (src < lo_excl[3]), 2, -1)),
    )
    # NOTE: bkt from searchsorted gives the HIGHER window for overlap srcs
    # (src >= W[k+1] -> k+1); treat it as the upper candidate, oclass k means
    # flexible between buckets k and k+1
    cell = core * NGROUPS + grp
    ncell = N_CORES * NGROUPS
    fixed = np.zeros((ncell, NBKT), np.int64)
    ov = np.zeros((ncell, 3), np.int64)
    is_flex = oclass >= 0
    low_bkt = np.where(is_flex, oclass, bkt)
    np.add.at(fixed, (cell[~is_flex], bkt[~is_flex]), 1)
    np.add.at(ov, (cell[is_flex], oclass[is_flex]), 1)
    tot = fixed.sum(1) + ov.sum(1)
    # measured per-queue gen-end offsets at equal counts (+2.0/2.4/2.3us for
    # q1-3 vs q0: dispatch stagger + per-queue ucode fixed cost); shift edges
    # so the four pairs' generation END times align instead of their counts
    DELTA = (206.0, -42.0, -86.0, -78.0)
    a01 = np.clip(
        np.rint(tot / 4 + DELTA[0] - fixed[:, 0]).astype(np.int64), 0, ov[:, 0]
    )
    n0 = fixed[:, 0] + a01
    a12 = np.clip(
        np.rint(tot / 2 + DELTA[0] + DELTA[1] - n0 - fixed[:, 1] - (ov[:, 0] - a01))
        .astype(np.int64),
        0, ov[:, 1],
    )
    n1 = fixed[:, 1] + (ov[:, 0] - a01) + a12
    a23 = np.clip(
        np.rint(
            3 * tot / 4 + DELTA[0] + DELTA[1] + DELTA[2]
            - n0 - n1 - fixed[:, 2] - (ov[:, 1] - a12)
        ).astype(np.int64),
        0, ov[:, 2],
    )
    take_low = np.stack([a01, a12, a23], axis=1)  # flex edges going to bucket k
    # rank each flexible edge within its (cell, oclass) pool
    fkey = cell * 3 + oclass
    fkey_masked = np.where(is_flex, fkey, -1)
    order_f = np.argsort(fkey_masked, kind="stable")
    nflex_lead = E - is_flex.sum()
    rank = np.empty(E, np.int64)
    counts_f = np.bincount(fkey_masked[is_flex], minlength=ncell * 3)
    starts_f = np.zeros(ncell * 3 + 1, np.int64)
    np.cumsum(counts_f, out=starts_f[1:])
    pos_f = np.arange(E, dtype=np.int64) - nflex_lead
    rank[order_f] = pos_f - np.where(
        fkey_masked[order_f] >= 0, starts_f[np.maximum(fkey_masked[order_f], 0)], 0
    )
    goes_low = is_flex & (rank < take_low[cell, np.maximum(oclass, 0)])
    bkt = np.where(goes_low, np.maximum(oclass, 0), bkt)
    sloc = src - W[bkt]
    assert sloc.min() >= 0 and sloc.max() < WSIZE
    region = (core * NGROUPS + grp) * NBKT + bkt  # gather region id
    tot_reg = N_CORES * NGATH

    key = region * GROUP + half
    # sort each (region, block) segment by source row: the gather then walks
    # HBM in increasing address order, improving DRAM row locality
    order = np.lexsort((sloc, key))
    ks = key[order]
    counts_h = np.bincount(key, minlength=tot_reg * GROUP)
    starts_h = np.zeros(tot_reg * GROUP + 1, np.int64)
    np.cumsum(counts_h, out=starts_h[1:])
    within = np.arange(E, dtype=np.int64) - starts_h[ks]

    ch = counts_h.reshape(tot_reg, GROUP)
    pref = np.zeros((tot_reg, GROUP + 1), np.int64)
    np.cumsum(ch, axis=1, out=pref[:, 1:])
    cnt_tot = pref[:, GROUP]
    # slot within region: block-major packing
    slot = np.empty(E, np.int64)
    slot[order] = within + pref[ks // GROUP, ks % GROUP]

    # build-time template parameters (uniform across cores by construction)
    chunks_p = int(np.ceil(cnt_tot.max() / 128))
    pad_lo = int(cnt_tot.min() // 128)  # chunks < pad_lo are always gathered
    los = tuple(int(v) for v in (pref[:, :GROUP] // 128).min(axis=0))
    his = tuple(
        int(v) for v in (np.maximum(pref[:, 1:] - 1, pref[:, :GROUP]) // 128).max(axis=0)
    )
    capp = chunks_p * 128

    idx_arr = np.full(tot_reg * capp, -1, np.int16)
    idx_arr[region * capp + slot] = sloc.astype(np.int16)
    cnt_arr = np.ascontiguousarray(
        cnt_tot.reshape(N_CORES, 1, NGATH).astype(np.int32)
    )

    # dstv columns: per region, block h covers chunks [los[h], his[h]];
    # -5 where the slot isn't the column's block
    spans = [his[h] - los[h] + 1 for h in range(GROUP)]
    col_base = np.concatenate([[0], np.cumsum(spans)]).astype(np.int64)
    nc_pb = int(col_base[-1])
    chunk = slot // 128
    pos = slot % 128
    colidx = col_base[half] + (chunk - np.asarray(los)[half])
    dcol = region * nc_pb + colidx
    dst_arr = np.full((tot_reg * nc_pb, 128), -5.0, np.float32)
    dst_arr[dcol, pos] = dloc.astype(np.float32)

    # idx: logical slot i of a gather -> partition i%16, col i//16; tile 16->128
    idx_sb = (
        idx_arr.reshape(N_CORES, NGATH, capp // 16, 16)
        .transpose(0, 3, 1, 2)
        .reshape(N_CORES, 16, NGATH * (capp // 16))
    )
    idx_sb = np.ascontiguousarray(np.tile(idx_sb, (1, 8, 1)))
    # dstv: [core, 128 partitions, cols], then duplicate each column value
    # pairwise along the innermost axis for the DVE 2x_1p compare
    dst_sb = dst_arr.reshape(N_CORES, NGATH * nc_pb, 128).transpose(0, 2, 1)
    dst_sb = np.ascontiguousarray(np.repeat(dst_sb, 2, axis=2))
    return idx_sb, dst_sb, cnt_arr, chunks_p, pad_lo, los, his


def _run(x, src_idx, dst_idx, trace=False, trace_kwargs=None):
    import ml_dtypes
    from concourse import bass_utils

    bf16 = ml_dtypes.bfloat16
    x = np.ascontiguousarray(np.asarray(x, dtype=np.float32))
    idx_sb, dst_sb, cnt_arr, chunks_p, pad_lo, los, his = _preprocess(
        np.asarray(src_idx), np.asarray(dst_idx)
    )
    dst_sb = dst_sb.astype(bf16)

    tkey = (chunks_p, pad_lo, los, his)
    if _cached.get("key") != tkey:
        _cached["nc"] = _build_program(chunks_p, pad_lo, los, his)
        _cached["key"] = tkey
    nc = _cached["nc"]

    x_bf = x.astype(bf16)
    iota = np.tile(np.arange(128, dtype=np.float32), (128, 1)).astype(bf16)
    ident = np.zeros((128, BLOCK), dtype=np.float32)
    ident[np.arange(BLOCK), np.arange(BLOCK)] = 1.0
    ident = ident.astype(bf16)
    in_maps = []
    for c in range(N_CORES):
        in_maps.append(
            {
                "x": x_bf,
                "xresb": x_bf[c * NODES_PER_CORE : c * NODES_PER_CORE + NBLOCKS * BLOCK],
                "idx": idx_sb[c],
                "cnt": cnt_arr[c],
                "dstv": dst_sb[c],
                "iota": iota,
                "ident": ident,
            }
        )
    kw = dict(trace_kwargs or {})
    res = bass_utils.run_bass_kernel_spmd(
        nc, in_maps, core_ids=list(range(N_CORES)), trace=trace, **kw
    )
    out = np.concatenate([r["out"] for r in res.results], axis=0)
    return out, res


def kernel(x, src_idx, dst_idx):
    out, _ = _run(x, src_idx, dst_idx)
    return out


# revision 31
# speedup vs baseline: 1.2080x; 1.0709x over previous
"""HGNN message passing (gather + segment_sum + residual) on 8 trn2 cores.

out = x + segment_sum(x[src_idx], dst_idx, num_segments=N)

Strategy (node-sharded accumulation, no collectives):
  - dst nodes sharded across 8 cores (12500 nodes each); each core owns the
    edges targeting its node range and produces its [12500, 128] output slice.
  - Nodes are processed in GROUPS of 5 blocks of 125. Edges of a group are
    bucketed by src//25000 (4 buckets, int16 gather-offset reach) and packed
    tightly, block-major, with a -1 tail the Q7 gather kernel never touches
    (the count register carries the exact edge count); the 4 buckets' gathers
    run CONCURRENTLY on the 4 Q7 core-pairs (one SWDGE queue each), and the
    per-edge Q7 descriptor-generation rate (~8ns/edge/pair) is the kernel's
    hard bottleneck -- bigger groups amortize the per-instruction overhead
    (GROUP=5 beat 4; 10 loses more to SBUF-forced shallow buffering than it
    gains).
  - bf16 x rows (256B) are fetched with gpsimd dma_gather; 12 stage buffers
    (3 groups) plus 3 rotating count-register sets keep the in-order GpSimd
    engine fed (a shared register set would add a write-after-read edge
    from each group's reg_loads to the previous group's gathers, locking
    the pairs into group lockstep).
  - Per (group, bucket), ONE fused DVE is_equal builds the one-hot matrices
    from a host-prebuilt dstv image whose values are duplicated pairwise
    (innermost stride-1 pairs on both operands unlock the DVE 2x_1p perf
    mode -- twice the one-hot build rate of a stride-0 broadcast compare,
    and with no on-device expansion step the compare depends on nothing but
    its tile pool). The segment-sum is a sum of bf16 one-hot matmuls
    accumulated in PSUM, one PSUM tile per block. A chunk whose slot range
    can straddle a block boundary gets one matmul per candidate block (the
    host writes -5 into the other blocks' dstv so the one-hot is zero
    there). The residual enters the same PSUM accumulation as an
    identity-matrix matmul against the bf16 x row block.
  - The idx image streams through a small pool one (group, bucket) tile at
    a time instead of one huge upfront load (group 0's tiles load before
    the large dstv const so the first gathers start early); stage-tile
    zeroing runs on the otherwise-idle ACT engine at startup; residual
    loads land once per GROUP (one strided DMA for 5 blocks) on the sync
    engine's queue; output blocks are staged through SBUF by the Scalar
    (ACT) engine and written once per GROUP from ACT's HWDGE queue.

All cores run one SPMD program; per-core data differences live entirely in
the input tensors. The matmul template (chunk count, per-block chunk spans)
is computed from the actual edge data at build time, uniform across cores.
"""
import os

import numpy as np

N_NODES = 100000
D = 128
N_CORES = 8
NODES_PER_CORE = N_NODES // N_CORES  # 12500
BLOCK = 125
NBLOCKS = NODES_PER_CORE // BLOCK  # 100
if os.environ.get("KERNEL_NBLOCKS"):  # debug-only scale-down (multiple of GROUP)
    NBLOCKS = int(os.environ["KERNEL_NBLOCKS"])
GROUP = 5
NGROUPS = NBLOCKS // GROUP
NBKT = 4
# Overlapping 32768-row gather windows (int16 offset reach). ~31% of srcs
# fall in an overlap and can be assigned to either adjacent queue; the host
# uses them to equalize the 4 Q7 pairs' per-(core,group) edge counts, since
# the group makespan is the max of the 4 concurrent descriptor-gen times.
WBASE = (0, 22411, 44822, 67232)
WSIZE = 32768
NGATH = NGROUPS * NBKT  # gathers per core
STAGE_BUFS = 12

_cached = {}


def _build_program(chunks_p, pad_lo, los, his):
    """chunks_p: slots per (group,bucket) gather / 128; block h of a group
    only ever has edges in chunks [los[h], his[h]] (host-verified)."""
    from concourse import bacc, mybir, library_config
    import concourse.tile as tile

    capp = chunks_p * 128
    spans = [his[h] - los[h] + 1 for h in range(GROUP)]
    col_base = np.concatenate([[0], np.cumsum(spans)]).astype(int)
    nc_pb = int(col_base[-1])  # dstv/onehot columns per (group, bucket)
    idx_cols = NGATH * (capp // 16)
    gidx_cols = NBKT * (capp // 16)  # idx columns per group

    nc = bacc.Bacc("TRN2", debug=False, num_swdge_queues=4)
    f32 = mybir.dt.float32
    bf16 = mybir.dt.bfloat16
    x_t = nc.dram_tensor("x", [N_NODES, D], bf16, kind="ExternalInput")
    xresb_t = nc.dram_tensor("xresb", [NBLOCKS * BLOCK, D], bf16, kind="ExternalInput")
    idx_t = nc.dram_tensor("idx", [128, idx_cols], mybir.dt.int16, kind="ExternalInput")
    cnt_t = nc.dram_tensor("cnt", [1, NGATH], mybir.dt.int32, kind="ExternalInput")
    # dstv with every column value duplicated pairwise (see module docstring)
    dstv_t = nc.dram_tensor(
        "dstv", [128, NGATH * nc_pb * 2], bf16, kind="ExternalInput"
    )
    iota_t = nc.dram_tensor("iota", [128, 128], bf16, kind="ExternalInput")
    ident_t = nc.dram_tensor("ident", [128, BLOCK], bf16, kind="ExternalInput")
    out_t = nc.dram_tensor("out", [NBLOCKS * BLOCK, D], f32, kind="ExternalOutput")

    with tile.TileContext(nc) as tc:
        with (
            tc.tile_pool(name="consts", bufs=1) as constp,
            tc.tile_pool(name="idxp", bufs=16) as idxp,
            tc.tile_pool(name="stage", bufs=STAGE_BUFS) as stagep,
            tc.tile_pool(name="oh", bufs=8) as ohp,
            tc.tile_pool(name="psum", bufs=8, space="PSUM") as psump,
            tc.tile_pool(name="resid", bufs=3) as residp,
            tc.tile_pool(name="osb", bufs=3) as osbp,
        ):
            nc.gpsimd.load_library(library_config.mlp)
            # cnt + first idx group load first: the first gathers depend on
            # them, everything else can trail
            cnt_sb = constp.tile([1, NGATH], mybir.dt.int32)
            nc.sync.dma_start(cnt_sb[:], cnt_t[:])
            # 3 rotating register sets: reg_load(g+1,k) would otherwise have
            # a register WAR on gather(g,k), serializing the Q7 pair
            # pipeline into lockstep groups
            NREGSET = 3
            cnt_regs = [
                nc.gpsimd.alloc_register(f"cnt{r}_{k}")
                for r in range(NREGSET)
                for k in range(NBKT)
            ]
            # group 0's idx image loads before the big dstv const so the
            # first gathers are not stuck behind it on the sync queue
            idx_g0 = []
            for k in range(NBKT):
                t = idxp.tile([128, capp // 16], mybir.dt.int16)
                nc.sync.dma_start(t[:], idx_t[:, k * (capp // 16) : (k + 1) * (capp // 16)])
                idx_g0.append(t)
            dstv_sb = constp.tile([128, NGATH * nc_pb * 2], bf16)
            nc.sync.dma_start(dstv_sb[:], dstv_t[:])
            iota_sb = constp.tile([128, 128], bf16)
            nc.sync.dma_start(iota_sb[:], iota_t[:])
            ident_sb = constp.tile([128, BLOCK], bf16)
            nc.sync.dma_start(ident_sb[:], ident_t[:])

            # zero staging once: stale SBUF may hold NaN bit patterns, and
            # NaN * 0 would poison the PSUM accumulation. Run on ACT, which
            # is idle at startup (DVE must not delay the first IS_EQs, and
            # gpsimd must not delay the first gathers).
            # only chunks >= pad_lo can ever hold boot-stale (possibly
            # NaN-patterned) slots that a matmul later reads through a zero
            # one-hot; chunks below are overwritten by every gather
            for _ in range(STAGE_BUFS):
                stage = stagep.tile([128, chunks_p, D], bf16)
                nc.scalar.memzero(stage[:, pad_lo:, :])

            iota_b = (
                iota_sb[:]
                .rearrange("p (a b) -> p a b", b=2)
                .unsqueeze(1)
                .broadcast_to([128, nc_pb, 64, 2])
            )

            for grp in range(NGROUPS):
                # per-bucket idx tiles: each gather depends only on its own
                # quarter of the group's idx image, so the first gathers can
                # start as soon as their slice lands
                if grp == 0:
                    idx_gk = idx_g0
                else:
                    idx_gk = []
                    for k in range(NBKT):
                        t = idxp.tile([128, capp // 16], mybir.dt.int16)
                        base = grp * gidx_cols + k * (capp // 16)
                        nc.sync.dma_start(t[:], idx_t[:, base : base + capp // 16])
                        idx_gk.append(t)
                stages_g = []
                ohbs_g = []
                # one batched register load for the group's 4 counts, then
                # the 4 gathers dispatch back-to-back on the engine (each
                # runs on its own Q7 pair; interleaved reg_loads would add
                # dispatch serialization between the pairs' starts)
                gregs = [cnt_regs[(grp % NREGSET) * NBKT + k] for k in range(NBKT)]
                nc.gpsimd.reg_load(gregs, cnt_sb[:1, grp * NBKT : (grp + 1) * NBKT])
                for k in range(NBKT):
                    stage = stagep.tile([128, chunks_p, D], bf16)
                    nc.gpsimd.dma_gather(
                        stage[:],
                        x_t[WBASE[k] : WBASE[k] + WSIZE, :],
                        idx_gk[k][:],
                        capp,
                        gregs[k],
                        D,
                        single_packet=False,
                        queue_num=k,
                    )
                    stages_g.append(stage)
                    g = grp * NBKT + k
                    ohb = ohp.tile([128, nc_pb, 128], bf16)
                    nc.vector.tensor_tensor(
                        ohb[:].rearrange("p c (a b) -> p c a b", b=2),
                        dstv_sb[:, g * nc_pb * 2 : (g + 1) * nc_pb * 2]
                        .rearrange("p (c b) -> p c b", b=2)
                        .unsqueeze(2)
                        .broadcast_to([128, nc_pb, 64, 2]),
                        iota_b,
                        mybir.AluOpType.is_equal,
                    )
                    ohbs_g.append(ohb)

                # one strided DMA loads the whole group's residual rows
                resid = residp.tile([BLOCK, GROUP, D], bf16)
                nc.sync.dma_start(
                    resid[:],
                    xresb_t[grp * GROUP * BLOCK : (grp + 1) * GROUP * BLOCK].rearrange(
                        "(h p) d -> p h d", h=GROUP
                    ),
                )
                osb = osbp.tile([BLOCK, GROUP, D], f32)
                for h in range(GROUP):
                    psum = psump.tile([BLOCK, D], f32, space="PSUM")
                    nc.tensor.matmul(
                        out=psum[:],
                        lhsT=ident_sb[:BLOCK, :],
                        rhs=resid[:, h, :],
                        start=True,
                        stop=False,
                    )
                    ncols_h = his[h] - los[h] + 1
                    for k in range(NBKT):
                        for i in range(ncols_h):
                            nc.tensor.matmul(
                                out=psum[:],
                                lhsT=ohbs_g[k][:, int(col_base[h]) + i, :BLOCK],
                                rhs=stages_g[k][:, los[h] + i, :],
                                start=False,
                                stop=(k == NBKT - 1 and i == ncols_h - 1),
                            )
                    nc.scalar.copy(osb[:, h, :], psum[:])
                # one strided DMA writes the whole group's output, issued
                # from the ACT engine's HWDGE queue
                nc.scalar.dma_start(
                    out_t[grp * GROUP * BLOCK : (grp + 1) * GROUP * BLOCK].rearrange(
                        "(h p) d -> p h d", h=GROUP
                    ),
                    osb[:],
                )

    nc.compile()
    return nc


def _preprocess(src, dst):
    """Pack edges into tight per-(group,bucket) gather regions; build the idx
    image, exact counts, and the dstv one-hot source columns."""
    src = src.astype(np.int64)
    dst = dst.astype(np.int64)
    if NBLOCKS < NODES_PER_CORE // BLOCK:  # debug: drop edges past the cut
        keep = (dst % NODES_PER_CORE) // BLOCK < NBLOCKS
        src, dst = src[keep], dst[keep]
    E = src.shape[0]
    core = dst // NODES_PER_CORE
    blk = (dst % NODES_PER_CORE) // BLOCK
    half = blk % GROUP
    grp = blk // GROUP
    dloc = (dst % NODES_PER_CORE) % BLOCK
    # balanced bucket assignment over the overlapping windows: srcs covered
    # by two windows are waterfilled so each (core, group) cell's 4 bucket
    # counts are as equal as possible (the Q7 pairs run in lockstep per
    # group, so the makespan is the max of the 4 counts)
    W = np.asarray(WBASE, np.int64)
    lo_excl = np.array([0, WBASE[0] + WSIZE, WBASE[1] + WSIZE, WBASE[2] + WSIZE])
    bkt = np.searchsorted(W[1:], src, side="right")  # lowest covering window
    oclass = np.where(
        (src >= W[1]) & (src < lo_excl[1]), 0,
        np.where((src >= W[2]) & (src < lo_excl[2]), 1,
                 np.where((src >= W[3]) & (src < lo_excl[3]), 2, -1)),
    )
    # NOTE: bkt from searchsorted gives the HIGHER window for overlap srcs
    # (src >= W[k+1] -> k+1); treat it as the upper candidate, oclass k means
    # flexible between buckets k and k+1
    cell = core * NGROUPS + grp
    ncell = N_CORES * NGROUPS
    fixed = np.zeros((ncell, NBKT), np.int64)
    ov = np.zeros((ncell, 3), np.int64)
    is_flex = oclass >= 0
    low_bkt = np.where(is_flex, oclass, bkt)
    np.add.at(fixed, (cell[~is_flex], bkt[~is_flex]), 1)
    np.add.at(ov, (cell[is_flex], oclass[is_flex]), 1)
    tot = fixed.sum(1) + ov.sum(1)
    # measured per-queue gen-end offsets at equal counts (+2.0/2.4/2.3us for
    # q1-3 vs q0: dispatch stagger + per-queue ucode fixed cost); shift edges
    # so the four pairs' generation END times align instead of their counts
    DELTA = (0.0, 0.0, 0.0, 0.0)
    a01 = np.clip(
        np.rint(tot / 4 + DELTA[0] - fixed[:, 0]).astype(np.int64), 0, ov[:, 0]
    )
    n0 = fixed[:, 0] + a01
    a12 = np.clip(
        np.rint(tot / 2 + DELTA[0] + DELTA[1] - n0 - fixed[:, 1] - (ov[:, 0] - a01))
        .astype(np.int64),
        0, ov[:, 1],
    )
    n1 = fixed[:, 1] + (ov[:, 0] - a01) + a12
    a23 = np.clip(
        np.rint(
            3 * tot / 4 + DELTA[0] + DELTA[1] + DELTA[2]
            - n0 - n1 - fixed[:, 2] - (ov[:, 1] - a12)
        ).astype(np.int64),
        0, ov[:, 2],
    )
    take_low = np.stack([a01, a12, a23], axis=1)  # flex edges going to bucket k
    # rank each flexible edge within its (cell, oclass) pool
    fkey = cell * 3 + oclass
    fkey_masked = np.where(is_flex, fkey, -1)
    order_f = np.argsort(fkey_masked, kind="stable")
    nflex_lead = E - is_flex.sum()
    rank = np.empty(E, np.int64)
    counts_f = np.bincount(fkey_masked[is_flex], minlength=ncell * 3)
    starts_f = np.zeros(ncell * 3 + 1, np.int64)
    np.cumsum(counts_f, out=starts_f[1:])
    pos_f = np.arange(E, dtype=np.int64) - nflex_lead
    rank[order_f] = pos_f - np.where(
        fkey_masked[order_f] >= 0, starts_f[np.maximum(fkey_masked[order_f], 0)], 0
    )
    goes_low = is_flex & (rank < take_low[cell, np.maximum(oclass, 0)])
    bkt = np.where(goes_low, np.maximum(oclass, 0), bkt)
    sloc = src - W[bkt]
    assert sloc.min() >= 0 and sloc.max() < WSIZE
    region = (core * NGROUPS + grp) * NBKT + bkt  # gather region id
    tot_reg = N_CORES * NGATH

    key = region * GROUP + half
    # sort each (region, block) segment by source row: the gather then walks
    # HBM in increasing address order, improving DRAM row locality
    order = np.lexsort((sloc, key))
    ks = key[order]
    counts_h = np.bincount(key, minlength=tot_reg * GROUP)
    starts_h = np.zeros(tot_reg * GROUP + 1, np.int64)
    np.cumsum(counts_h, out=starts_h[1:])
    within = np.arange(E, dtype=np.int64) - starts_h[ks]

    ch = counts_h.reshape(tot_reg, GROUP)
    pref = np.zeros((tot_reg, GROUP + 1), np.int64)
    np.cumsum(ch, axis=1, out=pref[:, 1:])
    cnt_tot = pref[:, GROUP]
    # slot within region: block-major packing
    slot = np.empty(E, np.int64)
    slot[order] = within + pref[ks // GROUP, ks % GROUP]

    # build-time template parameters (uniform across cores by construction)
    chunks_p = int(np.ceil(cnt_tot.max() / 128))
    pad_lo = int(cnt_tot.min() // 128)  # chunks < pad_lo are always gathered
    los = tuple(int(v) for v in (pref[:, :GROUP] // 128).min(axis=0))
    his = tuple(
        int(v) for v in (np.maximum(pref[:, 1:] - 1, pref[:, :GROUP]) // 128).max(axis=0)
    )
    capp = chunks_p * 128

    idx_arr = np.full(tot_reg * capp, -1, np.int16)
    idx_arr[region * capp + slot] = sloc.astype(np.int16)
    cnt_arr = np.ascontiguousarray(
        cnt_tot.reshape(N_CORES, 1, NGATH).astype(np.int32)
    )

    # dstv columns: per region, block h covers chunks [los[h], his[h]];
    # -5 where the slot isn't the column's block
    spans = [his[h] - los[h] + 1 for h in range(GROUP)]
    col_base = np.concatenate([[0], np.cumsum(spans)]).astype(np.int64)
    nc_pb = int(col_base[-1])
    chunk = slot // 128
    pos = slot % 128
    colidx = col_base[half] + (chunk - np.asarray(los)[half])
    dcol = region * nc_pb + colidx
    dst_arr = np.full((tot_reg * nc_pb, 128), -5.0, np.float32)
    dst_arr[dcol, pos] = dloc.astype(np.float32)

    # idx: logical slot i of a gather -> partition i%16, col i//16; tile 16->128
    idx_sb = (
        idx_arr.reshape(N_CORES, NGATH, capp // 16, 16)
        .transpose(0, 3, 1, 2)
        .reshape(N_CORES, 16, NGATH * (capp // 16))
    )
    idx_sb = np.ascontiguousarray(np.tile(idx_sb, (1, 8, 1)))
    # dstv: [core, 128 partitions, cols], then duplicate each column value
    # pairwise along the innermost axis for the DVE 2x_1p compare
    dst_sb = dst_arr.reshape(N_CORES, NGATH * nc_pb, 128).transpose(0, 2, 1)
    dst_sb = np.ascontiguousarray(np.repeat(dst_sb, 2, axis=2))
    return idx_sb, dst_sb, cnt_arr, chunks_p, pad_lo, los, his


def _run(x, src_idx, dst_idx, trace=False, trace_kwargs=None):
    import ml_dtypes
    from concourse import bass_utils

    bf16 = ml_dtypes.bfloat16
    x = np.ascontiguousarray(np.asarray(x, dtype=np.float32))
    idx_sb, dst_sb, cnt_arr, chunks_p, pad_lo, los, his = _preprocess(
        np.asarray(src_idx), np.asarray(dst_idx)
    )
    dst_sb = dst_sb.astype(bf16)

    tkey = (chunks_p, pad_lo, los, his)
    if _cached.get("key") != tkey:
        _cached["nc"] = _build_program(chunks_p, pad_lo, los, his)
        _cached["key"] = tkey
    nc = _cached["nc"]

    x_bf = x.astype(bf16)
    iota = np.tile(np.arange(128, dtype=np.float32), (128, 1)).astype(bf16)
    ident = np.zeros((128, BLOCK), dtype=np.float32)
    ident[np.arange(BLOCK), np.arange(BLOCK)] = 1.0
    ident = ident.astype(bf16)
    in_maps = []
    for c in range(N_CORES):
        in_maps.append(
            {
                "x": x_bf,
                "xresb": x_bf[c * NODES_PER_CORE : c * NODES_PER_CORE + NBLOCKS * BLOCK],
                "idx": idx_sb[c],
                "cnt": cnt_arr[c],
                "dstv": dst_sb[c],
                "iota": iota,
                "ident": ident,
            }
        )
    kw = dict(trace_kwargs or {})
    res = bass_utils.run_bass_kernel_spmd(
        nc, in_maps, core_ids=list(range(N_CORES)), trace=trace, **kw
    )
    out = np.concatenate([r["out"] for r in res.results], axis=0)
    return out, res


def kernel(x, src_idx, dst_idx):
    out, _ = _run(x, src_idx, dst_idx)
    return out
